# revision 1
# baseline (speedup 1.0000x reference)
"""BasicTransDecoderBlock on 8 Trainium2 NeuronCores.

Strategy: the 4-head attention over 4096 queries x 4096 keys dominates
(~99% of FLOPs and intermediate traffic). It is sharded query-wise across
the 8 cores (512 queries each, all 4 heads per core). The rel-pos bias
idx is affine: idx[i,j] = u(i)-u(j)+480 (mod 29791) with u=31h+w+d in
[0,495], and each core's queries span only 62 consecutive u values, so
the whole per-core bias reduces to a (4096 keys x 62) table read through
a strided access pattern - no per-element gather.  exp(s*(qk+bias)) =
exp(s*qk) * exp(s*bias): ACT does the exp from PSUM, DVE multiplies by
the exp-bias table (bf16, 2x mode), PE does QK and AV.  Softmax
normalization is folded into the AV matmul via an appended ones column
(row 32 of the output = sum of weights), with the final divide done on
the host during unsharding.  The small convs / BN / interpolations are
cheap glue computed on the host.
"""

import sys
import numpy as np

sys.path.insert(0, "/opt/trn_rl_repo")

import ml_dtypes

IN_CH, OUT_CH, HEADS, DIM_HEAD, R = 256, 128, 4, 32, 16
EPS = 1e-5
SCALE = DIM_HEAD ** -0.5
N = R * R * R          # 4096 keys / queries
QPC = N // 8           # 512 queries per core
CWIN = 62              # distinct u values per core (2 h-planes)
CPAD = 64              # padded bias window stride
VPAD = 36              # 32 dims + 1 ones col, padded


# ---------------- host-side numpy reference pieces ----------------

def _pw(x, w):
    b, c = x.shape[0], x.shape[1]
    xf = x.reshape(b, c, -1)
    o = np.einsum("oi,bif->bof", w.reshape(w.shape[0], c), xf)
    return o.reshape(b, w.shape[0], *x.shape[2:])


def _dw(x, wd):
    b, c, h, w, d = x.shape
    xp = np.zeros((b, c, h + 2, w + 2, d + 2), x.dtype)
    xp[:, :, 1:-1, 1:-1, 1:-1] = x
    out = np.zeros_like(x)
    for a in range(3):
        for bb in range(3):
            for cc in range(3):
                out += wd[None, :, 0, a, bb, cc, None, None, None] * \
                    xp[:, :, a:a + h, bb:bb + w, cc:cc + d]
    return out


def _bn(x, g, b):
    m = x.mean(axis=(0, 2, 3, 4), keepdims=True, dtype=np.float32)
    v = ((x - m) ** 2).mean(axis=(0, 2, 3, 4), keepdims=True, dtype=np.float32)
    return (x - m) / np.sqrt(v + EPS) * g.reshape(1, -1, 1, 1, 1) + \
        b.reshape(1, -1, 1, 1, 1)


def _interp1(x, axis, out_len):
    in_len = x.shape[axis]
    if in_len == out_len:
        return x
    pos = np.arange(out_len, dtype=x.dtype) * ((in_len - 1) / (out_len - 1))
    lo = np.clip(np.floor(pos).astype(np.int32), 0, in_len - 1)
    hi = np.clip(lo + 1, 0, in_len - 1)
    w = (pos - lo.astype(x.dtype))
    shp = [1] * x.ndim
    shp[axis] = out_len
    w = w.reshape(shp)
    return np.take(x, lo, axis=axis) * (1 - w) + np.take(x, hi, axis=axis) * w


def _interp3(x, size):
    for ax, s in zip((2, 3, 4), size):
        x = _interp1(x, ax, s)
    return x


def _u_vec():
    hh, ww, dd = np.meshgrid(np.arange(R), np.arange(R), np.arange(R),
                             indexing="ij")
    return (31 * hh + ww + dd).reshape(-1)  # (4096,), u in [0,495]


def _t_ext(rel_table):
    # t_ext[h, m] for m in [-15, 975] stored at index m+15 -> length 991
    m = np.arange(-15, 976) % ((2 * R - 1) ** 3)
    return rel_table[m, :].T.astype(np.float32)  # (4, 991)


# ---------------- device kernel ----------------

_CACHE = {}


def _ap4(t, ap_dims):
    import concourse.bass as bass
    b = t if isinstance(t, bass.AP) else t[:]
    return bass.AP(tensor=b.tensor, offset=b.offset,
                   ap=[list(b.ap[0])] + ap_dims)


def _build_bass():
    import concourse.bass as bass
    import concourse.mybir as mybir
    from contextlib import ExitStack

    dt = mybir.dt
    nc = bass.Bass()
    kT = nc.dram_tensor("kT", [HEADS, 32, N], dt.float32, kind="ExternalInput")
    qT = nc.dram_tensor("qT", [HEADS, 32, QPC], dt.float32, kind="ExternalInput")
    VA = nc.dram_tensor("VA", [128, HEADS * 32 * VPAD], dt.bfloat16,
                        kind="ExternalInput")
    WE = nc.dram_tensor("WE", [128, HEADS * 32 * CPAD], dt.bfloat16,
                        kind="ExternalInput")
    OT = nc.dram_tensor("OT", [HEADS, VPAD, QPC], dt.float32,
                        kind="ExternalOutput")

    T = HEADS * 32  # 128 pipeline steps
    NB = 2          # double buffering

    with ExitStack() as ctx:
        en = ctx.enter_context
        kT_sb = en(nc.sbuf_tensor("kT_sb", [32, HEADS * N], dt.float32))
        qT_sb = en(nc.sbuf_tensor("qT_sb", [32, HEADS * QPC], dt.float32))
        va_sb = en(nc.sbuf_tensor("va_sb", [128, HEADS * 32 * VPAD], dt.bfloat16))
        we_sb = en(nc.sbuf_tensor("we_sb", [128, HEADS * 32 * CPAD], dt.bfloat16))
        e_sb = [en(nc.sbuf_tensor(f"e_sb{i}", [128, QPC], dt.bfloat16)) for i in range(NB)]
        p_sb = [en(nc.sbuf_tensor(f"p_sb{i}", [128, QPC], dt.bfloat16)) for i in range(NB)]
        ob_sb = [en(nc.sbuf_tensor(f"ob_sb{i}", [VPAD, QPC], dt.float32)) for i in range(HEADS)]
        pq_ps = [en(nc.psum_tensor(f"pq_ps{i}", [128, QPC], dt.float32)) for i in range(NB)]
        po_ps = [en(nc.psum_tensor(f"po_ps{i}", [VPAD, QPC], dt.float32)) for i in range(HEADS)]

        dmas = en(nc.semaphore("dmas"))
        qks = en(nc.semaphore("qks"))
        acts = en(nc.semaphore("acts"))
        dvs = en(nc.semaphore("dvs"))
        avs = en(nc.semaphore("avs"))
        cps = en(nc.semaphore("cps"))
        blk = en(nc.Block())

        NDMA = 2 * HEADS + 2

        @blk.sync
        def _(s):
            for h in range(HEADS):
                s.dma_start(kT_sb[:, h * N:(h + 1) * N], kT[h]).then_inc(dmas, 16)
                s.dma_start(qT_sb[:, h * QPC:(h + 1) * QPC], qT[h]).then_inc(dmas, 16)
            s.dma_start(va_sb[:], VA[:]).then_inc(dmas, 16)
            s.dma_start(we_sb[:], WE[:]).then_inc(dmas, 16)
            for h in range(HEADS):
                s.wait_ge(cps, h + 1)
                s.dma_start(OT[h], ob_sb[h][:]).then_inc(dmas, 16)

        @blk.tensor
        def _(t):
            t.wait_ge(dmas, 16 * NDMA)
            for ti in range(T):
                h, jc = ti // 32, ti % 32
                if ti >= 2:
                    t.wait_ge(acts, ti - 1)
                t.matmul(pq_ps[ti % NB][:],
                         kT_sb[:, h * N + jc * 128: h * N + (jc + 1) * 128],
                         qT_sb[:, h * QPC:(h + 1) * QPC],
                         start=True, stop=True).then_inc(qks, 1)
                if ti >= 1:
                    tp = ti - 1
                    hp, jp = tp // 32, tp % 32
                    t.wait_ge(dvs, tp + 1)
                    t.matmul(po_ps[hp][:],
                             va_sb[:, tp * VPAD:(tp + 1) * VPAD],
                             p_sb[tp % NB][:],
                             start=(jp == 0), stop=(jp == 31)).then_inc(avs, 1)
            tp = T - 1
            t.wait_ge(dvs, tp + 1)
            t.matmul(po_ps[HEADS - 1][:],
                     va_sb[:, tp * VPAD:(tp + 1) * VPAD],
                     p_sb[tp % NB][:],
                     start=False, stop=True).then_inc(avs, 1)

        @blk.scalar
        def _(s):
            for ti in range(T):
                s.wait_ge(qks, ti + 1)
                if ti >= NB:
                    s.wait_ge(dvs, ti - 1)
                s.activation(e_sb[ti % NB][:], pq_ps[ti % NB][:],
                             mybir.ActivationFunctionType.Exp,
                             scale=float(SCALE)).then_inc(acts, 1)

        @blk.vector
        def _(v):
            v.wait_ge(dmas, 16 * NDMA)
            for ti in range(T):
                v.wait_ge(acts, ti + 1)
                if ti >= NB:
                    v.wait_ge(avs, ti - 1)
                base = we_sb[:, ti * CPAD: ti * CPAD + CWIN]
                w_ap = _ap4(base, [[31, 2], [1, 16], [1, 16]])
                e4 = _ap4(e_sb[ti % NB], [[256, 2], [16, 16], [1, 16]])
                p4 = _ap4(p_sb[ti % NB], [[256, 2], [16, 16], [1, 16]])
                v.tensor_tensor(p4, e4, w_ap,
                                op=mybir.AluOpType.mult).then_inc(dvs, 1)
            for h in range(HEADS):
                v.wait_ge(avs, 32 * (h + 1))
                v.tensor_copy(ob_sb[h][:], po_ps[h][:]).then_inc(cps, 1)
    return nc


def _device_attention(qh, kh, vh, rel_table):
    """qh/kh/vh: (4, 4096, 32) f32. Returns o (4, 4096, 32) f32 normalized."""
    from concourse.bass_utils import run_bass_kernel_spmd

    if "nc" not in _CACHE:
        _CACHE["nc"] = _build_bass()
    nc = _CACHE["nc"]

    u = _u_vec()
    te = _t_ext(rel_table)          # (4, 991), index m+15
    bf16 = ml_dtypes.bfloat16

    kT = np.ascontiguousarray(kh.transpose(0, 2, 1))          # (4,32,4096)
    qT_all = np.ascontiguousarray(qh.transpose(0, 2, 1))      # (4,32,4096)

    # v augmented: (128 part, 4*32*VPAD)
    va4 = np.zeros((HEADS, 32, 128, VPAD), np.float32)
    va4[:, :, :, :32] = vh.reshape(HEADS, 32, 128, 32)
    va4[:, :, :, 32] = 1.0
    va = np.ascontiguousarray(
        va4.transpose(2, 0, 1, 3).reshape(128, -1)).astype(bf16)

    tabs = np.exp(SCALE * te)                                  # (4, 991)
    cc = np.arange(CWIN)
    base_midx = cc[None, :] + 495 - u[:, None]                 # (4096, 62)
    in_maps = []
    for c in range(8):
        # exp-bias window table: WE[p, (h*32+jc)*CPAD + cc]
        #   = exp(SCALE * t_ext[h, cc + 62c - u_j + 480]),  j = jc*128+p
        g = tabs[:, base_midx + 62 * c]                        # (4, 4096, 62)
        we4 = np.zeros((HEADS, 32, 128, CPAD), np.float32)
        we4[:, :, :, :CWIN] = g.reshape(HEADS, 32, 128, CWIN)
        we = np.ascontiguousarray(
            we4.transpose(2, 0, 1, 3).reshape(128, -1)).astype(bf16)
        in_maps.append({
            "kT": kT,
            "qT": np.ascontiguousarray(qT_all[:, :, c * QPC:(c + 1) * QPC]),
            "VA": va,
            "WE": we,
        })

    import kernel as _self
    try:
        res = run_bass_kernel_spmd(nc, in_maps, list(range(8)),
                                   trace=bool(_CACHE.get("trace")))
        if getattr(res, "exec_time_ns", None):
            _self._LAST_EXEC_NS = res.exec_time_ns
    except Exception:
        res = run_bass_kernel_spmd(nc, in_maps, list(range(8)))
    o = np.zeros((HEADS, N, 32), np.float32)
    for c in range(8):
        ot = res.results[c]["OT"]                              # (4, VPAD, 512)
        for h in range(HEADS):
            z = ot[h, 32, :]                                   # (512,)
            o[h, c * QPC:(c + 1) * QPC, :] = (ot[h, :32, :] / z[None, :]).T
    return o


def _host_attention(qh, kh, vh, rel_table):
    u = _u_vec()
    te = _t_ext(rel_table)
    o = np.zeros((HEADS, N, 32), np.float32)
    m = u[:, None] - u[None, :] + 480 + 15                     # (4096,4096)
    for h in range(HEADS):
        bias = te[h][m]
        logits = (qh[h] @ kh[h].T + bias) * SCALE
        logits -= logits.max(axis=-1, keepdims=True)
        p = np.exp(logits)
        p /= p.sum(axis=-1, keepdims=True)
        o[h] = p @ vh[h]
    return o


# ---------------- main entry ----------------

def kernel(x1, x2, w_ch, b_ch, gamma_l, beta_l, gamma_h, beta_h, gamma2,
           beta2, kv_dw, kv_pw, q_dw, q_pw, out_dw, out_pw, w_mlp, rel_table):
    x1 = np.asarray(x1, np.float32)
    x2 = np.asarray(x2, np.float32)
    rel_table = np.asarray(rel_table, np.float32)

    HH = x2.shape[2]
    residue = _interp3(_pw(x1, np.asarray(w_ch, np.float32)) +
                       np.asarray(b_ch, np.float32).reshape(1, -1, 1, 1, 1),
                       (HH, HH, HH))
    x1n = _bn(x1, np.asarray(gamma_l, np.float32), np.asarray(beta_l, np.float32))
    x2n = _bn(x2, np.asarray(gamma_h, np.float32), np.asarray(beta_h, np.float32))
    kv = _pw(_dw(x1n, np.asarray(kv_dw, np.float32)), np.asarray(kv_pw, np.float32))
    k_, v_ = kv[:, :OUT_CH], kv[:, OUT_CH:]
    q_ = _pw(_dw(x2n, np.asarray(q_dw, np.float32)), np.asarray(q_pw, np.float32))
    k_ = _interp3(k_, (R, R, R))
    v_ = _interp3(v_, (R, R, R))

    def heads_split(t):
        # channel c = dd*HEADS + h
        b, c = t.shape[0], t.shape[1]
        t = t.reshape(b, DIM_HEAD, HEADS, -1)        # (1,32,4,4096)
        return np.ascontiguousarray(t[0].transpose(1, 2, 0))  # (4,4096,32)

    qh, kh, vh = heads_split(q_), heads_split(k_), heads_split(v_)

    try:
        o = _device_attention(qh, kh, vh, rel_table)
    except Exception as exc:  # insurance: keep output correct
        print(f"[kernel] device path failed ({exc!r}); numpy fallback",
              file=sys.stderr)
        o = _host_attention(qh, kh, vh, rel_table)

    # reassemble channels: o_full[dd*4+h, i] = o[h, i, dd]
    o_full = np.zeros((OUT_CH, N), np.float32)
    for h in range(HEADS):
        o_full[h::HEADS, :] = o[h].T
    o_sp = o_full.reshape(1, OUT_CH, R, R, R)

    o1 = _pw(_dw(o_sp, np.asarray(out_dw, np.float32)),
             np.asarray(out_pw, np.float32))
    o1 = o1 + residue
    res2 = o1
    o2 = np.maximum(_bn(o1, np.asarray(gamma2, np.float32),
                        np.asarray(beta2, np.float32)), 0.0)
    o3 = _pw(o2, np.asarray(w_mlp, np.float32))
    return (o3 + res2).astype(np.float32)



# revision 2
# speedup vs baseline: 7.4757x; 7.4757x over previous
"""BasicTransDecoderBlock on Trainium2 — head-sharded attention, v3.

The 4-head 4096x4096x32 attention dominates. It is sharded one head per
core on 4 cores: no k/v replication, so bytes through the ~50MB/s axon
tunnel are minimal — that transfer is the entire cost of this problem.

v3 additions over the query-sharded v1:
 - k/v ship PRE-interpolation ([32 x 512] bf16 each instead of the 8x
   expanded forms); the align-corners trilinear 8->16 interp runs on
   device as 3 axis passes of 16 two-tap DVE slice ops each (f32
   intermediates), with v then transposed to key-partition layout via 32
   identity matmuls.
 - the exp'd rel-pos bias windowed table WE (2MB/core) is built on
   device from a [32 x 971] pre-shifted table slice (62KB) with PE
   permutation matmuls: WE = S @ etabR-window, S[r,p] = 1 iff
   r = p//16 + p%16 (the only non-affine part of the index map).
 - no donated zero output buffers (kernel writes every output element).
 - host glue (convs/BN/interp) uses torch (single thread) when
   available; numpy otherwise.

Execution: module-cached jax.jit(shard_map(bass_exec)) — no per-call
retrace — falling back to run_bass_kernel_spmd, then to pure numpy.
"""

import sys
import numpy as np

sys.path.insert(0, "/opt/trn_rl_repo")

import ml_dtypes

BF16 = ml_dtypes.bfloat16
IN_CH, OUT_CH, HEADS, DIM_HEAD, R = 256, 128, 4, 32, 16
EPS = 1e-5
SCALE = DIM_HEAD ** -0.5
N = R * R * R           # 4096 keys / queries
NBLK = 8                # 512-query blocks per core
QB = 512
CWIN = 62               # distinct u values per 512-query block
CPAD = 64               # padded per-(jc) stride in WE
VPAD = 36               # 32 dims + 1 ones col, padded
ETW = 971               # etabR free width
NCORE = 4

# align-corners 8->16 interp taps: out[j] = in[lo[j]]*(1-w[j]) + in[hi[j]]*w[j]
_ILO = [min(int(j * 7 / 15), 7) for j in range(16)]
_IW = [float(np.float32(j * (7.0 / 15.0)) - np.float32(l))
       for j, l in zip(range(16), _ILO)]
_IHI = [min(l + 1, 7) for l in _ILO]

try:
    import torch
    import torch.nn.functional as _TF
    torch.set_num_threads(1)
    _TORCH = True
except Exception:
    _TORCH = False


# ---------------- host-side glue (torch fast path / numpy fallback) ----

def _pw(x, w):
    c = x.shape[1]
    o = w.reshape(w.shape[0], c) @ x.reshape(c, -1)
    return o.reshape(1, w.shape[0], *x.shape[2:])


def _dw_np(x, wd):
    b, c, h, w, d = x.shape
    xp = np.zeros((c, h + 2, w + 2, d + 2), x.dtype)
    xp[:, 1:-1, 1:-1, 1:-1] = x[0]
    out = np.zeros((c, h, w, d), x.dtype)
    tmp = np.empty_like(out)
    wf = wd[:, 0]
    for a in range(3):
        for bb in range(3):
            for cc in range(3):
                np.multiply(xp[:, a:a + h, bb:bb + w, cc:cc + d],
                            wf[:, a, bb, cc, None, None, None], out=tmp)
                out += tmp
    return out[None]


def _dw(x, wd):
    if _TORCH:
        return _TF.conv3d(torch.from_numpy(np.ascontiguousarray(x)),
                          torch.from_numpy(np.ascontiguousarray(wd)),
                          padding=1, groups=x.shape[1]).numpy()
    return _dw_np(x, wd)


def _bn(x, g, b):
    m = x.mean(axis=(0, 2, 3, 4), keepdims=True, dtype=np.float32)
    v = ((x - m) ** 2).mean(axis=(0, 2, 3, 4), keepdims=True, dtype=np.float32)
    return (x - m) / np.sqrt(v + EPS) * g.reshape(1, -1, 1, 1, 1) + \
        b.reshape(1, -1, 1, 1, 1)


def _interp1(x, axis, out_len):
    in_len = x.shape[axis]
    if in_len == out_len:
        return x
    pos = np.arange(out_len, dtype=x.dtype) * ((in_len - 1) / (out_len - 1))
    lo = np.clip(np.floor(pos).astype(np.int32), 0, in_len - 1)
    hi = np.clip(lo + 1, 0, in_len - 1)
    w = (pos - lo.astype(x.dtype))
    shp = [1] * x.ndim
    shp[axis] = out_len
    w = w.reshape(shp)
    return np.take(x, lo, axis=axis) * (1 - w) + np.take(x, hi, axis=axis) * w


def _interp3(x, size):
    if _TORCH:
        return _TF.interpolate(torch.from_numpy(np.ascontiguousarray(x)),
                               size=size, mode="trilinear",
                               align_corners=True).numpy()
    for ax, s in zip((2, 3, 4), size):
        x = _interp1(x, ax, s)
    return x


def _u_vec():
    hh, ww, dd = np.meshgrid(np.arange(R), np.arange(R), np.arange(R),
                             indexing="ij")
    return (31 * hh + ww + dd).reshape(-1)


# ---------------- device kernel ----------------

_CACHE = {}


def _ap4(t, ap_dims):
    import concourse.bass as bass
    b = t if isinstance(t, bass.AP) else t[:]
    return bass.AP(tensor=b.tensor, offset=b.offset,
                   ap=[list(b.ap[0])] + ap_dims)


def _build_bass():
    import concourse.bass as bass  # noqa: F401
    import concourse.mybir as mybir
    from contextlib import ExitStack

    dt = mybir.dt
    nc = bass.Bass()
    KS = nc.dram_tensor("KS", [32, 512], dt.bfloat16, kind="ExternalInput")
    VS = nc.dram_tensor("VS", [32, 512], dt.bfloat16, kind="ExternalInput")
    QT = nc.dram_tensor("QT", [32, N], dt.bfloat16, kind="ExternalInput")
    ETR = nc.dram_tensor("ETR", [32, ETW], dt.bfloat16, kind="ExternalInput")
    SP = nc.dram_tensor("SP", [32, 128], dt.bfloat16, kind="ExternalInput")
    IDT = nc.dram_tensor("IDT", [32, 32], dt.float32, kind="ExternalInput")
    OT = nc.dram_tensor("OT", [NBLK, 33, QB], dt.bfloat16, kind="ExternalOutput")

    T = NBLK * 32  # 256 pipeline steps
    NB = 2

    with ExitStack() as ctx:
        en = ctx.enter_context
        ks_sb = en(nc.sbuf_tensor("ks_sb", [32, 512], dt.bfloat16))
        vs_sb = en(nc.sbuf_tensor("vs_sb", [32, 512], dt.bfloat16))
        qT_sb = en(nc.sbuf_tensor("qT_sb", [32, N], dt.bfloat16))
        etr_sb = en(nc.sbuf_tensor("etr_sb", [32, ETW], dt.bfloat16))
        sp_sb = en(nc.sbuf_tensor("sp_sb", [32, 128], dt.bfloat16))
        idt_sb = en(nc.sbuf_tensor("idt_sb", [32, 32], dt.float32))
        i1_sb = en(nc.sbuf_tensor("i1_sb", [32, 1024], dt.float32))
        i2_sb = en(nc.sbuf_tensor("i2_sb", [32, 2048], dt.float32))
        kT_sb = en(nc.sbuf_tensor("kT_sb", [32, N], dt.bfloat16))
        vf_sb = en(nc.sbuf_tensor("vf_sb", [32, N], dt.float32))
        tmp_sb = en(nc.sbuf_tensor("tmp_sb", [32, 256], dt.float32))
        tm2_sb = en(nc.sbuf_tensor("tm2_sb", [32, 256], dt.float32))
        va_sb = en(nc.sbuf_tensor("va_sb", [128, 32 * VPAD], dt.bfloat16))
        we_sb = en(nc.sbuf_tensor("we_sb", [128, NBLK * 32 * CPAD], dt.bfloat16))
        e_sb = [en(nc.sbuf_tensor(f"e_sb{i}", [128, QB], dt.bfloat16)) for i in range(NB)]
        p_sb = [en(nc.sbuf_tensor(f"p_sb{i}", [128, QB], dt.bfloat16)) for i in range(NB)]
        ob_sb = [en(nc.sbuf_tensor(f"ob_sb{i}", [33, QB], dt.bfloat16)) for i in range(NBLK)]
        exp_ps = en(nc.psum_tensor("exp_ps", [128, 1024], dt.float32))
        tr_ps = en(nc.psum_tensor("tr_ps", [128, 32], dt.float32))
        pq_ps = [en(nc.psum_tensor(f"pq_ps{i}", [128, QB], dt.float32)) for i in range(NB)]
        po_ps = [en(nc.psum_tensor(f"po_ps{i}", [VPAD, QB], dt.float32)) for i in range(NB)]

        dmas = en(nc.semaphore("dmas"))
        expm = en(nc.semaphore("expm"))
        expc = en(nc.semaphore("expc"))
        vrdy = en(nc.semaphore("vrdy"))
        krdy = en(nc.semaphore("krdy"))
        trm = en(nc.semaphore("trm"))
        trc = en(nc.semaphore("trc"))
        qks = en(nc.semaphore("qks"))
        acts = en(nc.semaphore("acts"))
        dvs = en(nc.semaphore("dvs"))
        avs = en(nc.semaphore("avs"))
        cps = en(nc.semaphore("cps"))
        blk = en(nc.Block())

        NDMA_IN = 6

        def interp_axis(v, src, dst, ostr, istr, splane, dplane, sem=None):
            # dst[:, J*ostr + dplane] = src[:, lo*istr + splane]*(1-w)
            #                         + src[:, hi*istr + splane]*w
            last = None
            nel = 1
            for st, ct in splane:
                nel *= ct
            for j in range(16):
                lo, hi, w = _ILO[j], _IHI[j], _IW[j]
                sl_lo = _ap4(src[:, lo * istr:lo * istr + 1], splane)
                sl_hi = _ap4(src[:, hi * istr:hi * istr + 1], splane)
                sl_o = _ap4(dst[:, j * ostr:j * ostr + 1], dplane)
                sl_t = _ap4(tmp_sb[:, 0:1], [[1, nel]])
                sl_t2 = _ap4(tm2_sb[:, 0:1], [[1, nel]])
                v.tensor_scalar_mul(sl_t, sl_lo, 1.0 - w)
                v.tensor_scalar_mul(sl_t2, sl_hi, w)
                last = v.tensor_tensor(sl_o, sl_t, sl_t2,
                                       op=mybir.AluOpType.add)
            if sem is not None:
                last.then_inc(sem, 1)

        @blk.sync
        def _(s):
            s.dma_start(ks_sb[:], KS[:]).then_inc(dmas, 16)
            s.dma_start(vs_sb[:], VS[:]).then_inc(dmas, 16)
            s.dma_start(qT_sb[:], QT[:]).then_inc(dmas, 16)
            s.dma_start(etr_sb[:], ETR[:]).then_inc(dmas, 16)
            s.dma_start(sp_sb[:], SP[:]).then_inc(dmas, 16)
            s.dma_start(idt_sb[:], IDT[:]).then_inc(dmas, 16)
            for g in range(NBLK):
                s.wait_ge(cps, g + 1)
                s.dma_start(OT[g], ob_sb[g][:]).then_inc(dmas, 16)

        @blk.tensor
        def _(t):
            t.wait_ge(dmas, 16 * NDMA_IN)
            # --- WE expand: 32 matmuls (4 per block), psum holds 2 chunks ---
            for mi in range(NBLK * 4):
                g, qt = mi // 4, mi % 4
                if mi >= 2:
                    t.wait_ge(expc, mi // 2)
                off = 497 - 62 * g + 31 * 4 * qt
                rhs = _ap4(etr_sb[:, off:off + CWIN],
                           [[31, 4], [8, 2], [-1, CWIN]])
                outap = _ap4(exp_ps[:, 512 * (mi % 2):512 * (mi % 2) + CWIN],
                             [[128, 4], [64, 2], [1, CWIN]])
                t.matmul(outap, sp_sb[:], rhs, start=True,
                         stop=True).then_inc(expm, 1)
            # --- v transpose to key-partition layout: 32 identity mms ---
            t.wait_ge(vrdy, 1)
            for jc in range(32):
                if jc >= 1:
                    t.wait_ge(trc, jc)
                t.matmul(tr_ps[:], vf_sb[:, jc * 128:(jc + 1) * 128],
                         idt_sb[:], start=True, stop=True).then_inc(trm, 1)
            # --- main attention pipeline ---
            t.wait_ge(krdy, 1)
            for ti in range(T):
                g, jc = ti // 32, ti % 32
                if ti >= 2:
                    t.wait_ge(acts, ti - 1)
                t.matmul(pq_ps[ti % NB][:],
                         kT_sb[:, jc * 128:(jc + 1) * 128],
                         qT_sb[:, g * QB:(g + 1) * QB],
                         start=True, stop=True).then_inc(qks, 1)
                if ti >= 1:
                    tp = ti - 1
                    gp, jp = tp // 32, tp % 32
                    t.wait_ge(dvs, tp + 1)
                    if jp == 0 and gp >= 2:
                        t.wait_ge(cps, gp - 1)
                    t.matmul(po_ps[gp % NB][:],
                             va_sb[:, jp * VPAD:(jp + 1) * VPAD],
                             p_sb[tp % NB][:],
                             start=(jp == 0), stop=(jp == 31)).then_inc(avs, 1)
            tp = T - 1
            t.wait_ge(dvs, tp + 1)
            t.matmul(po_ps[(tp // 32) % NB][:],
                     va_sb[:, 31 * VPAD:32 * VPAD],
                     p_sb[tp % NB][:],
                     start=False, stop=True).then_inc(avs, 1)

        @blk.scalar
        def _(s):
            for ti in range(T):
                s.wait_ge(qks, ti + 1)
                if ti >= NB:
                    s.wait_ge(dvs, ti - 1)
                s.activation(e_sb[ti % NB][:], pq_ps[ti % NB][:],
                             mybir.ActivationFunctionType.Exp,
                             scale=float(SCALE)).then_inc(acts, 1)

        @blk.vector
        def _(v):
            v.wait_ge(dmas, 16 * NDMA_IN)
            # ones for the VA norm column (and its pad)
            v.memset(va_sb[:], 1.0)
            # --- v interp: (x8,y8,z8) -> (X16,Y16,Z16), f32 intermediates ---
            # i1: (X16,y8,z8) strides (64,8,1); i2: (X16,Y16,z8) (128,8,1)
            interp_axis(v, vs_sb, i1_sb, 64, 64,
                        [[8, 8], [1, 8]], [[8, 8], [1, 8]])
            interp_axis(v, i1_sb, i2_sb, 8, 8,
                        [[64, 16], [1, 8]], [[128, 16], [1, 8]])
            interp_axis(v, i2_sb, vf_sb, 1, 1,
                        [[128, 16], [8, 16]], [[256, 16], [16, 16]],
                        sem=vrdy)
            # --- k interp ---
            interp_axis(v, ks_sb, i1_sb, 64, 64,
                        [[8, 8], [1, 8]], [[8, 8], [1, 8]])
            interp_axis(v, i1_sb, i2_sb, 8, 8,
                        [[64, 16], [1, 8]], [[128, 16], [1, 8]])
            interp_axis(v, i2_sb, kT_sb, 1, 1,
                        [[128, 16], [8, 16]], [[256, 16], [16, 16]],
                        sem=krdy)
            # --- WE expand copies: 16 x [128, 1024] psum->bf16 ---
            for ci in range(16):
                v.wait_ge(expm, 2 * (ci + 1))
                v.tensor_copy(we_sb[:, ci * 1024:(ci + 1) * 1024],
                              exp_ps[:]).then_inc(expc, 1)
            # --- va copies from transpose psum ---
            for jc in range(32):
                v.wait_ge(trm, jc + 1)
                v.tensor_copy(va_sb[:, jc * VPAD:jc * VPAD + 32],
                              tr_ps[:]).then_inc(trc, 1)
            # --- main multiplies + per-block output copies ---
            for ti in range(T):
                g, jc = ti // 32, ti % 32
                v.wait_ge(acts, ti + 1)
                if ti >= NB:
                    v.wait_ge(avs, ti - 1)
                base = we_sb[:, (g * 32 + jc) * CPAD: (g * 32 + jc) * CPAD + CWIN]
                w_ap = _ap4(base, [[31, 2], [1, 16], [1, 16]])
                e4 = _ap4(e_sb[ti % NB], [[256, 2], [16, 16], [1, 16]])
                p4 = _ap4(p_sb[ti % NB], [[256, 2], [16, 16], [1, 16]])
                v.tensor_tensor(p4, e4, w_ap,
                                op=mybir.AluOpType.mult).then_inc(dvs, 1)
                if jc == 31:
                    v.wait_ge(avs, 32 * (g + 1))
                    v.tensor_copy(ob_sb[g][:],
                                  po_ps[g % NB][0:33, :]).then_inc(cps, 1)
    return nc


def _exp_table(rel_table):
    """exptab[h, M] = exp(SCALE * T[(M - 15) % 29791]) for M in [0, 999)."""
    m = (np.arange(999) - 15) % ((2 * R - 1) ** 3)
    return np.exp(SCALE * rel_table[m, :].astype(np.float32)).T  # (4, 999)


def _build_in_maps(qh, ksm, vsm, rel_table):
    """qh: (4, 4096, 32); ksm/vsm: (4, 32, 512) pre-interp per-head k/v."""
    exptab = _exp_table(rel_table)
    idx = np.clip(np.arange(ETW)[None, :] + np.arange(32)[:, None] - 2, 0, 998)
    pv = np.arange(128)
    S = np.zeros((32, 128), np.float32)
    S[pv // 16 + pv % 16, pv] = 1.0
    S = S.astype(BF16)
    I32 = np.eye(32, dtype=np.float32)
    in_maps = []
    for h in range(HEADS):
        in_maps.append({
            "KS": ksm[h].astype(BF16),
            "VS": vsm[h].astype(BF16),
            "QT": np.ascontiguousarray(qh[h].T).astype(BF16),
            "ETR": exptab[h][idx].astype(BF16),
            "SP": S,
            "IDT": I32,
        })
    return in_maps


def _unshard(results):
    o = np.zeros((HEADS, N, 32), np.float32)
    for h in range(HEADS):
        ot = results[h]["OT"].astype(np.float32)          # (8, 33, 512)
        o[h] = (ot[:, :32, :] / ot[:, 32:33, :]).transpose(0, 2, 1).reshape(N, 32)
    return o


def _get_runner():
    """Cached jax.jit(shard_map(bass_exec)) over 4 cores, no zero-donation."""
    if "runner" in _CACHE:
        return _CACHE["runner"]
    import jax
    import concourse.mybir as mybir
    from jax.sharding import Mesh, PartitionSpec
    from jax.experimental.shard_map import shard_map
    from concourse.bass2jax import (_bass_exec_p, install_neuronx_cc_hook,
                                    partition_id_tensor)

    nc = _CACHE.get("nc")
    if nc is None:
        nc = _CACHE["nc"] = _build_bass()
    install_neuronx_cc_hook()

    partition_name = (nc.partition_id_tensor.name
                      if nc.partition_id_tensor else None)
    in_names, out_names, out_avals = [], [], []
    for alloc in nc.m.functions[0].allocations:
        if not isinstance(alloc, mybir.MemoryLocationSet):
            continue
        name = alloc.memorylocations[0].name
        if alloc.kind == "ExternalInput":
            if name != partition_name:
                in_names.append(name)
        elif alloc.kind == "ExternalOutput":
            out_names.append(name)
            out_avals.append(jax.core.ShapedArray(
                tuple(alloc.tensor_shape), mybir.dt.np(alloc.dtype)))
    in_names_full = tuple(in_names) + (
        (partition_name,) if partition_name else ())

    def _body(*args):
        operands = list(args)
        if partition_name is not None:
            operands.append(partition_id_tensor())
        outs = _bass_exec_p.bind(
            *operands, out_avals=tuple(out_avals), in_names=in_names_full,
            out_names=tuple(out_names), lowering_input_output_aliases=(),
            sim_require_finite=True, sim_require_nnan=True, nc=nc)
        return tuple(outs)

    devices = jax.devices()[:NCORE]
    mesh = Mesh(np.asarray(devices), ("core",))
    sharded = jax.jit(
        shard_map(_body, mesh=mesh,
                  in_specs=(PartitionSpec("core"),) * len(in_names),
                  out_specs=(PartitionSpec("core"),) * len(out_names),
                  check_rep=False),
        keep_unused=True)

    def run(in_maps):
        concat_in = [np.concatenate([m[name] for m in in_maps], axis=0)
                     for name in in_names]
        out_arrs = sharded(*concat_in)
        return [
            {name: np.asarray(out_arrs[i]).reshape(NCORE, *out_avals[i].shape)[c]
             for i, name in enumerate(out_names)}
            for c in range(NCORE)
        ]

    _CACHE["runner"] = run
    return run


def _device_attention(qh, ksm, vsm, rel_table):
    in_maps = _build_in_maps(qh, ksm, vsm, rel_table)
    try:
        run = _get_runner()
        results = run(in_maps)
    except Exception as exc:
        print(f"[kernel] cached runner failed ({exc!r}); spmd fallback",
              file=sys.stderr)
        from concourse.bass_utils import run_bass_kernel_spmd
        if "nc" not in _CACHE:
            _CACHE["nc"] = _build_bass()
        res = run_bass_kernel_spmd(_CACHE["nc"], in_maps, list(range(NCORE)))
        results = res.results
    return _unshard(results)


def _host_attention(qh, ksm, vsm, rel_table):
    # expand k/v on host (reference interp), then exact softmax attention
    u = _u_vec()
    exptab = _exp_table(rel_table)
    kh = np.zeros((HEADS, N, 32), np.float32)
    vh = np.zeros((HEADS, N, 32), np.float32)
    for h in range(HEADS):
        k3 = ksm[h].astype(np.float32).reshape(1, 32, 8, 8, 8)
        v3 = vsm[h].astype(np.float32).reshape(1, 32, 8, 8, 8)
        kh[h] = _interp3(k3, (R, R, R)).reshape(32, N).T
        vh[h] = _interp3(v3, (R, R, R)).reshape(32, N).T
    o = np.zeros((HEADS, N, 32), np.float32)
    m = u[:, None] - u[None, :] + 480 + 15
    for h in range(HEADS):
        logits = (qh[h] @ kh[h].T) * SCALE
        logits = logits + np.log(exptab[h])[m.T]
        logits -= logits.max(axis=-1, keepdims=True)
        p = np.exp(logits)
        p /= p.sum(axis=-1, keepdims=True)
        o[h] = p @ vh[h]
    return o


# ---------------- main entry ----------------

def kernel(x1, x2, w_ch, b_ch, gamma_l, beta_l, gamma_h, beta_h, gamma2,
           beta2, kv_dw, kv_pw, q_dw, q_pw, out_dw, out_pw, w_mlp, rel_table):
    x1 = np.asarray(x1, np.float32)
    x2 = np.asarray(x2, np.float32)
    rel_table = np.asarray(rel_table, np.float32)

    HH = x2.shape[2]
    residue = _interp3(_pw(x1, np.asarray(w_ch, np.float32)) +
                       np.asarray(b_ch, np.float32).reshape(1, -1, 1, 1, 1),
                       (HH, HH, HH))
    x1n = _bn(x1, np.asarray(gamma_l, np.float32), np.asarray(beta_l, np.float32))
    x2n = _bn(x2, np.asarray(gamma_h, np.float32), np.asarray(beta_h, np.float32))
    kv = _pw(_dw(x1n, np.asarray(kv_dw, np.float32)), np.asarray(kv_pw, np.float32))
    k_, v_ = kv[:, :OUT_CH], kv[:, OUT_CH:]
    q_ = _pw(_dw(x2n, np.asarray(q_dw, np.float32)), np.asarray(q_pw, np.float32))

    def heads_split(t):
        # channel c = dd*HEADS + h -> (heads, space, dim)
        t = t.reshape(DIM_HEAD, HEADS, -1)
        return np.ascontiguousarray(t.transpose(1, 2, 0))

    qh = heads_split(q_.reshape(OUT_CH, -1))
    # pre-interp per-head k/v: (4, 32, 512), layout [dim, (x,y,z) 8^3]
    ksm = np.ascontiguousarray(
        k_.reshape(DIM_HEAD, HEADS, 512).transpose(1, 0, 2))
    vsm = np.ascontiguousarray(
        v_.reshape(DIM_HEAD, HEADS, 512).transpose(1, 0, 2))

    try:
        o = _device_attention(qh, ksm, vsm, rel_table)
    except Exception as exc:  # insurance: keep output correct
        print(f"[kernel] device path failed ({exc!r}); numpy fallback",
              file=sys.stderr)
        o = _host_attention(qh, ksm, vsm, rel_table)

    # reassemble channels: o_full[dd*4+h, i] = o[h, i, dd]
    o_full = np.zeros((OUT_CH, N), np.float32)
    for h in range(HEADS):
        o_full[h::HEADS, :] = o[h].T
    o_sp = o_full.reshape(1, OUT_CH, R, R, R)

    o1 = _pw(_dw(o_sp, np.asarray(out_dw, np.float32)),
             np.asarray(out_pw, np.float32))
    o1 = o1 + residue
    res2 = o1
    o2 = np.maximum(_bn(o1, np.asarray(gamma2, np.float32),
                        np.asarray(beta2, np.float32)), 0.0)
    o3 = _pw(o2, np.asarray(w_mlp, np.float32))
    return (o3 + res2).astype(np.float32)


# revision 14
# speedup vs baseline: 8.9337x; 1.1950x over previous
"""BasicTransDecoderBlock on Trainium2 — head-sharded attention, v3.

The 4-head 4096x4096x32 attention dominates. It is sharded one head per
core on 4 cores: no k/v replication, so bytes through the ~50MB/s axon
tunnel are minimal — that transfer is the entire cost of this problem.

v3 additions over the query-sharded v1:
 - k/v ship PRE-interpolation ([32 x 512] bf16 each instead of the 8x
   expanded forms); the align-corners trilinear 8->16 interp runs on
   device as 3 axis passes of 16 two-tap DVE slice ops each (f32
   intermediates), with v then transposed to key-partition layout via 32
   identity matmuls.
 - the exp'd rel-pos bias windowed table WE (2MB/core) is built on
   device from a [32 x 971] pre-shifted table slice (62KB) with PE
   permutation matmuls: WE = S @ etabR-window, S[r,p] = 1 iff
   r = p//16 + p%16 (the only non-affine part of the index map).
 - no donated zero output buffers (kernel writes every output element).
 - host glue (convs/BN/interp) uses torch (single thread) when
   available; numpy otherwise.

Execution: module-cached jax.jit(shard_map(bass_exec)) — no per-call
retrace — falling back to run_bass_kernel_spmd, then to pure numpy.
"""

import sys
import numpy as np

sys.path.insert(0, "/opt/trn_rl_repo")

import ml_dtypes

BF16 = ml_dtypes.bfloat16
IN_CH, OUT_CH, HEADS, DIM_HEAD, R = 256, 128, 4, 32, 16
EPS = 1e-5
SCALE = DIM_HEAD ** -0.5
N = R * R * R           # 4096 keys / queries
NBLK = 8                # 512-query blocks per core
QB = 512
CWIN = 62               # distinct u values per 512-query block
CPAD = 64               # padded per-(jc) stride in WE
VPAD = 36               # 32 dims + 1 ones col, padded
ETW = 971               # etabR free width
NCORE = 4

# align-corners 8->16 interp taps: out[j] = in[lo[j]]*(1-w[j]) + in[hi[j]]*w[j]
_ILO = [min(int(j * 7 / 15), 7) for j in range(16)]
_IW = [float(np.float32(j * (7.0 / 15.0)) - np.float32(l))
       for j, l in zip(range(16), _ILO)]
_IHI = [min(l + 1, 7) for l in _ILO]

try:
    import torch
    import torch.nn.functional as _TF
    torch.set_num_threads(1)
    _TORCH = True
except Exception:
    _TORCH = False


# ---------------- host-side glue (torch fast path / numpy fallback) ----

def _pw(x, w):
    c = x.shape[1]
    o = w.reshape(w.shape[0], c) @ x.reshape(c, -1)
    return o.reshape(1, w.shape[0], *x.shape[2:])


def _dw_np(x, wd):
    b, c, h, w, d = x.shape
    xp = np.zeros((c, h + 2, w + 2, d + 2), x.dtype)
    xp[:, 1:-1, 1:-1, 1:-1] = x[0]
    out = np.zeros((c, h, w, d), x.dtype)
    tmp = np.empty_like(out)
    wf = wd[:, 0]
    for a in range(3):
        for bb in range(3):
            for cc in range(3):
                np.multiply(xp[:, a:a + h, bb:bb + w, cc:cc + d],
                            wf[:, a, bb, cc, None, None, None], out=tmp)
                out += tmp
    return out[None]


def _dw(x, wd):
    if _TORCH:
        return _TF.conv3d(torch.from_numpy(np.ascontiguousarray(x)),
                          torch.from_numpy(np.ascontiguousarray(wd)),
                          padding=1, groups=x.shape[1]).numpy()
    return _dw_np(x, wd)


def _bn(x, g, b):
    m = x.mean(axis=(0, 2, 3, 4), keepdims=True, dtype=np.float32)
    v = ((x - m) ** 2).mean(axis=(0, 2, 3, 4), keepdims=True, dtype=np.float32)
    return (x - m) / np.sqrt(v + EPS) * g.reshape(1, -1, 1, 1, 1) + \
        b.reshape(1, -1, 1, 1, 1)


def _interp1(x, axis, out_len):
    in_len = x.shape[axis]
    if in_len == out_len:
        return x
    pos = np.arange(out_len, dtype=x.dtype) * ((in_len - 1) / (out_len - 1))
    lo = np.clip(np.floor(pos).astype(np.int32), 0, in_len - 1)
    hi = np.clip(lo + 1, 0, in_len - 1)
    w = (pos - lo.astype(x.dtype))
    shp = [1] * x.ndim
    shp[axis] = out_len
    w = w.reshape(shp)
    return np.take(x, lo, axis=axis) * (1 - w) + np.take(x, hi, axis=axis) * w


def _interp3(x, size):
    # numpy beats torch interpolate at this size on 1 CPU
    for ax, s in zip((2, 3, 4), size):
        x = _interp1(x, ax, s)
    return x


def _u_vec():
    hh, ww, dd = np.meshgrid(np.arange(R), np.arange(R), np.arange(R),
                             indexing="ij")
    return (31 * hh + ww + dd).reshape(-1)


# ---------------- device kernel ----------------

_CACHE = {}


def _ap4(t, ap_dims):
    import concourse.bass as bass
    b = t if isinstance(t, bass.AP) else t[:]
    return bass.AP(tensor=b.tensor, offset=b.offset,
                   ap=[list(b.ap[0])] + ap_dims)


def _build_bass():
    import concourse.bass as bass  # noqa: F401
    import concourse.mybir as mybir
    from contextlib import ExitStack

    dt = mybir.dt
    nc = bass.Bass()
    KS = nc.dram_tensor("KS", [32, 512], dt.bfloat16, kind="ExternalInput")
    VS = nc.dram_tensor("VS", [32, 512], dt.bfloat16, kind="ExternalInput")
    QT = nc.dram_tensor("QT", [32, N], dt.bfloat16, kind="ExternalInput")
    ETR = nc.dram_tensor("ETR", [32, ETW], dt.bfloat16, kind="ExternalInput")
    SP = nc.dram_tensor("SP", [32, 128], dt.bfloat16, kind="ExternalInput")
    IDT = nc.dram_tensor("IDT", [32, 32], dt.float32, kind="ExternalInput")
    OT = nc.dram_tensor("OT", [NBLK, 33, QB], dt.bfloat16, kind="ExternalOutput")

    T = NBLK * 32  # 256 pipeline steps
    NB = 2

    with ExitStack() as ctx:
        en = ctx.enter_context
        ks_sb = en(nc.sbuf_tensor("ks_sb", [32, 512], dt.bfloat16))
        vs_sb = en(nc.sbuf_tensor("vs_sb", [32, 512], dt.bfloat16))
        qT_sb = en(nc.sbuf_tensor("qT_sb", [32, N], dt.bfloat16))
        etr_sb = en(nc.sbuf_tensor("etr_sb", [32, ETW], dt.bfloat16))
        sp_sb = en(nc.sbuf_tensor("sp_sb", [32, 128], dt.bfloat16))
        idt_sb = en(nc.sbuf_tensor("idt_sb", [32, 32], dt.float32))
        i1_sb = en(nc.sbuf_tensor("i1_sb", [32, 1024], dt.float32))
        i2_sb = en(nc.sbuf_tensor("i2_sb", [32, 2048], dt.float32))
        kT_sb = en(nc.sbuf_tensor("kT_sb", [32, N], dt.bfloat16))
        vf_sb = en(nc.sbuf_tensor("vf_sb", [32, N], dt.float32))
        tmp_sb = en(nc.sbuf_tensor("tmp_sb", [32, 256], dt.float32))
        tm2_sb = en(nc.sbuf_tensor("tm2_sb", [32, 256], dt.float32))
        va_sb = en(nc.sbuf_tensor("va_sb", [128, 32 * VPAD], dt.bfloat16))
        we_sb = en(nc.sbuf_tensor("we_sb", [128, NBLK * 32 * CPAD], dt.bfloat16))
        e_sb = [en(nc.sbuf_tensor(f"e_sb{i}", [128, QB], dt.bfloat16)) for i in range(NB)]
        p_sb = [en(nc.sbuf_tensor(f"p_sb{i}", [128, QB], dt.bfloat16)) for i in range(NB)]
        ob_sb = [en(nc.sbuf_tensor(f"ob_sb{i}", [33, QB], dt.bfloat16)) for i in range(NBLK)]
        exp_ps = en(nc.psum_tensor("exp_ps", [128, 1024], dt.float32))
        tr_ps = en(nc.psum_tensor("tr_ps", [128, 32], dt.float32))
        pq_ps = [en(nc.psum_tensor(f"pq_ps{i}", [128, QB], dt.float32)) for i in range(NB)]
        po_ps = [en(nc.psum_tensor(f"po_ps{i}", [VPAD, QB], dt.float32)) for i in range(NB)]

        dmas = en(nc.semaphore("dmas"))
        expm = en(nc.semaphore("expm"))
        expc = en(nc.semaphore("expc"))
        vrdy = en(nc.semaphore("vrdy"))
        krdy = en(nc.semaphore("krdy"))
        trm = en(nc.semaphore("trm"))
        trc = en(nc.semaphore("trc"))
        qks = en(nc.semaphore("qks"))
        acts = en(nc.semaphore("acts"))
        dvs = en(nc.semaphore("dvs"))
        avs = en(nc.semaphore("avs"))
        cps = en(nc.semaphore("cps"))
        blk = en(nc.Block())

        NDMA_IN = 6

        def interp_axis(v, src, dst, ostr, istr, splane, dplane, sem=None):
            # dst[:, J*ostr + dplane] = src[:, lo*istr + splane]*(1-w)
            #                         + src[:, hi*istr + splane]*w
            last = None
            nel = 1
            for st, ct in splane:
                nel *= ct
            for j in range(16):
                lo, hi, w = _ILO[j], _IHI[j], _IW[j]
                sl_lo = _ap4(src[:, lo * istr:lo * istr + 1], splane)
                sl_hi = _ap4(src[:, hi * istr:hi * istr + 1], splane)
                sl_o = _ap4(dst[:, j * ostr:j * ostr + 1], dplane)
                sl_t = _ap4(tmp_sb[:, 0:1], [[1, nel]])
                sl_t2 = _ap4(tm2_sb[:, 0:1], [[1, nel]])
                v.tensor_scalar_mul(sl_t, sl_lo, 1.0 - w)
                v.tensor_scalar_mul(sl_t2, sl_hi, w)
                last = v.tensor_tensor(sl_o, sl_t, sl_t2,
                                       op=mybir.AluOpType.add)
            if sem is not None:
                last.then_inc(sem, 1)

        @blk.sync
        def _(s):
            s.dma_start(ks_sb[:], KS[:]).then_inc(dmas, 16)
            s.dma_start(vs_sb[:], VS[:]).then_inc(dmas, 16)
            s.dma_start(qT_sb[:], QT[:]).then_inc(dmas, 16)
            s.dma_start(etr_sb[:], ETR[:]).then_inc(dmas, 16)
            s.dma_start(sp_sb[:], SP[:]).then_inc(dmas, 16)
            s.dma_start(idt_sb[:], IDT[:]).then_inc(dmas, 16)
            for g in range(NBLK):
                s.wait_ge(cps, g + 1)
                s.dma_start(OT[g], ob_sb[g][:]).then_inc(dmas, 16)

        @blk.tensor
        def _(t):
            t.wait_ge(dmas, 16 * NDMA_IN)
            # --- WE expand: 32 matmuls (4 per block), psum holds 2 chunks ---
            for mi in range(NBLK * 4):
                g, qt = mi // 4, mi % 4
                if mi >= 2:
                    t.wait_ge(expc, mi // 2)
                off = 497 - 62 * g + 31 * 4 * qt
                rhs = _ap4(etr_sb[:, off:off + CWIN],
                           [[31, 4], [8, 2], [-1, CWIN]])
                outap = _ap4(exp_ps[:, 512 * (mi % 2):512 * (mi % 2) + CWIN],
                             [[128, 4], [64, 2], [1, CWIN]])
                t.matmul(outap, sp_sb[:], rhs, start=True,
                         stop=True).then_inc(expm, 1)
            # --- v transpose to key-partition layout: 32 identity mms ---
            t.wait_ge(vrdy, 1)
            for jc in range(32):
                if jc >= 1:
                    t.wait_ge(trc, jc)
                t.matmul(tr_ps[:], vf_sb[:, jc * 128:(jc + 1) * 128],
                         idt_sb[:], start=True, stop=True).then_inc(trm, 1)
            # --- main attention pipeline ---
            t.wait_ge(krdy, 1)
            for ti in range(T):
                g, jc = ti // 32, ti % 32
                if ti >= 2:
                    t.wait_ge(acts, ti - 1)
                t.matmul(pq_ps[ti % NB][:],
                         kT_sb[:, jc * 128:(jc + 1) * 128],
                         qT_sb[:, g * QB:(g + 1) * QB],
                         start=True, stop=True).then_inc(qks, 1)
                if ti >= 1:
                    tp = ti - 1
                    gp, jp = tp // 32, tp % 32
                    t.wait_ge(dvs, tp + 1)
                    if jp == 0 and gp >= 2:
                        t.wait_ge(cps, gp - 1)
                    t.matmul(po_ps[gp % NB][:],
                             va_sb[:, jp * VPAD:(jp + 1) * VPAD],
                             p_sb[tp % NB][:],
                             start=(jp == 0), stop=(jp == 31)).then_inc(avs, 1)
            tp = T - 1
            t.wait_ge(dvs, tp + 1)
            t.matmul(po_ps[(tp // 32) % NB][:],
                     va_sb[:, 31 * VPAD:32 * VPAD],
                     p_sb[tp % NB][:],
                     start=False, stop=True).then_inc(avs, 1)

        @blk.scalar
        def _(s):
            for ti in range(T):
                s.wait_ge(qks, ti + 1)
                if ti >= NB:
                    s.wait_ge(dvs, ti - 1)
                s.activation(e_sb[ti % NB][:], pq_ps[ti % NB][:],
                             mybir.ActivationFunctionType.Exp,
                             scale=float(SCALE)).then_inc(acts, 1)

        @blk.vector
        def _(v):
            v.wait_ge(dmas, 16 * NDMA_IN)
            # ones for the VA norm column (and its pad)
            v.memset(va_sb[:], 1.0)
            # --- v interp: (x8,y8,z8) -> (X16,Y16,Z16), f32 intermediates ---
            # i1: (X16,y8,z8) strides (64,8,1); i2: (X16,Y16,z8) (128,8,1)
            interp_axis(v, vs_sb, i1_sb, 64, 64,
                        [[8, 8], [1, 8]], [[8, 8], [1, 8]])
            interp_axis(v, i1_sb, i2_sb, 8, 8,
                        [[64, 16], [1, 8]], [[128, 16], [1, 8]])
            interp_axis(v, i2_sb, vf_sb, 1, 1,
                        [[128, 16], [8, 16]], [[256, 16], [16, 16]],
                        sem=vrdy)
            # --- k interp ---
            interp_axis(v, ks_sb, i1_sb, 64, 64,
                        [[8, 8], [1, 8]], [[8, 8], [1, 8]])
            interp_axis(v, i1_sb, i2_sb, 8, 8,
                        [[64, 16], [1, 8]], [[128, 16], [1, 8]])
            interp_axis(v, i2_sb, kT_sb, 1, 1,
                        [[128, 16], [8, 16]], [[256, 16], [16, 16]],
                        sem=krdy)
            # --- WE expand copies: 16 x [128, 1024] psum->bf16 ---
            for ci in range(16):
                v.wait_ge(expm, 2 * (ci + 1))
                v.tensor_copy(we_sb[:, ci * 1024:(ci + 1) * 1024],
                              exp_ps[:]).then_inc(expc, 1)
            # --- va copies from transpose psum ---
            for jc in range(32):
                v.wait_ge(trm, jc + 1)
                v.tensor_copy(va_sb[:, jc * VPAD:jc * VPAD + 32],
                              tr_ps[:]).then_inc(trc, 1)
            # --- main multiplies + per-block output copies ---
            for ti in range(T):
                g, jc = ti // 32, ti % 32
                v.wait_ge(acts, ti + 1)
                if ti >= NB:
                    v.wait_ge(avs, ti - 1)
                base = we_sb[:, (g * 32 + jc) * CPAD: (g * 32 + jc) * CPAD + CWIN]
                w_ap = _ap4(base, [[31, 2], [1, 16], [1, 16]])
                e4 = _ap4(e_sb[ti % NB], [[256, 2], [16, 16], [1, 16]])
                p4 = _ap4(p_sb[ti % NB], [[256, 2], [16, 16], [1, 16]])
                v.tensor_tensor(p4, e4, w_ap,
                                op=mybir.AluOpType.mult).then_inc(dvs, 1)
                if jc == 31:
                    v.wait_ge(avs, 32 * (g + 1))
                    v.tensor_copy(ob_sb[g][:],
                                  po_ps[g % NB][0:33, :]).then_inc(cps, 1)
    return nc


def _exp_table(rel_table):
    """exptab[h, M] = exp(SCALE * T[(M - 15) % 29791]) for M in [0, 999)."""
    m = (np.arange(999) - 15) % ((2 * R - 1) ** 3)
    return np.exp(SCALE * rel_table[m, :].astype(np.float32)).T  # (4, 999)


def _build_in_maps(qh, ksm, vsm, rel_table):
    """qh: (4, 4096, 32); ksm/vsm: (4, 32, 512) pre-interp per-head k/v."""
    exptab = _exp_table(rel_table)
    idx = np.clip(np.arange(ETW)[None, :] + np.arange(32)[:, None] - 2, 0, 998)
    pv = np.arange(128)
    S = np.zeros((32, 128), np.float32)
    S[pv // 16 + pv % 16, pv] = 1.0
    S = S.astype(BF16)
    I32 = np.eye(32, dtype=np.float32)
    in_maps = []
    for h in range(HEADS):
        in_maps.append({
            "KS": ksm[h].astype(BF16),
            "VS": vsm[h].astype(BF16),
            "QT": np.ascontiguousarray(qh[h].T).astype(BF16),
            "ETR": exptab[h][idx].astype(BF16),
            "SP": S,
            "IDT": I32,
        })
    return in_maps


def _unshard(results):
    o = np.zeros((HEADS, N, 32), np.float32)
    for h in range(HEADS):
        ot = results[h]["OT"].astype(np.float32)          # (8, 33, 512)
        o[h] = (ot[:, :32, :] / ot[:, 32:33, :]).transpose(0, 2, 1).reshape(N, 32)
    return o


def _get_runner():
    """Cached jax.jit(shard_map(bass_exec)) over 4 cores, no zero-donation."""
    if "runner" in _CACHE:
        return _CACHE["runner"]
    import jax
    import concourse.mybir as mybir
    from jax.sharding import Mesh, PartitionSpec
    from jax.experimental.shard_map import shard_map
    from concourse.bass2jax import (_bass_exec_p, install_neuronx_cc_hook,
                                    partition_id_tensor)

    nc = _CACHE.get("nc")
    if nc is None:
        nc = _CACHE["nc"] = _build_bass()
    install_neuronx_cc_hook()

    partition_name = (nc.partition_id_tensor.name
                      if nc.partition_id_tensor else None)
    in_names, out_names, out_avals = [], [], []
    for alloc in nc.m.functions[0].allocations:
        if not isinstance(alloc, mybir.MemoryLocationSet):
            continue
        name = alloc.memorylocations[0].name
        if alloc.kind == "ExternalInput":
            if name != partition_name:
                in_names.append(name)
        elif alloc.kind == "ExternalOutput":
            out_names.append(name)
            out_avals.append(jax.core.ShapedArray(
                tuple(alloc.tensor_shape), mybir.dt.np(alloc.dtype)))
    in_names_full = tuple(in_names) + (
        (partition_name,) if partition_name else ())

    def _body(*args):
        operands = list(args)
        if partition_name is not None:
            operands.append(partition_id_tensor())
        outs = _bass_exec_p.bind(
            *operands, out_avals=tuple(out_avals), in_names=in_names_full,
            out_names=tuple(out_names), lowering_input_output_aliases=(),
            sim_require_finite=True, sim_require_nnan=True, nc=nc)
        return tuple(outs)

    devices = jax.devices()[:NCORE]
    mesh = Mesh(np.asarray(devices), ("core",))
    sharded = jax.jit(
        shard_map(_body, mesh=mesh,
                  in_specs=(PartitionSpec("core"),) * len(in_names),
                  out_specs=(PartitionSpec("core"),) * len(out_names),
                  check_rep=False),
        keep_unused=True)

    def run(in_maps):
        concat_in = [np.concatenate([m[name] for m in in_maps], axis=0)
                     for name in in_names]
        out_arrs = sharded(*concat_in)
        return [
            {name: np.asarray(out_arrs[i]).reshape(NCORE, *out_avals[i].shape)[c]
             for i, name in enumerate(out_names)}
            for c in range(NCORE)
        ]

    _CACHE["runner"] = run
    return run


def _device_attention(qh, ksm, vsm, rel_table):
    in_maps = _build_in_maps(qh, ksm, vsm, rel_table)
    try:
        run = _get_runner()
        results = run(in_maps)
    except Exception as exc:
        print(f"[kernel] cached runner failed ({exc!r}); spmd fallback",
              file=sys.stderr)
        from concourse.bass_utils import run_bass_kernel_spmd
        if "nc" not in _CACHE:
            _CACHE["nc"] = _build_bass()
        res = run_bass_kernel_spmd(_CACHE["nc"], in_maps, list(range(NCORE)))
        results = res.results
    return _unshard(results)


def _host_attention(qh, ksm, vsm, rel_table):
    # expand k/v on host (reference interp), then exact softmax attention
    u = _u_vec()
    exptab = _exp_table(rel_table)
    kh = np.zeros((HEADS, N, 32), np.float32)
    vh = np.zeros((HEADS, N, 32), np.float32)
    for h in range(HEADS):
        k3 = ksm[h].astype(np.float32).reshape(1, 32, 8, 8, 8)
        v3 = vsm[h].astype(np.float32).reshape(1, 32, 8, 8, 8)
        kh[h] = _interp3(k3, (R, R, R)).reshape(32, N).T
        vh[h] = _interp3(v3, (R, R, R)).reshape(32, N).T
    o = np.zeros((HEADS, N, 32), np.float32)
    m = u[:, None] - u[None, :] + 480 + 15
    for h in range(HEADS):
        logits = (qh[h] @ kh[h].T) * SCALE
        logits = logits + np.log(exptab[h])[m.T]
        logits -= logits.max(axis=-1, keepdims=True)
        p = np.exp(logits)
        p /= p.sum(axis=-1, keepdims=True)
        o[h] = p @ vh[h]
    return o


# ---------------- main entry ----------------

def kernel(x1, x2, w_ch, b_ch, gamma_l, beta_l, gamma_h, beta_h, gamma2,
           beta2, kv_dw, kv_pw, q_dw, q_pw, out_dw, out_pw, w_mlp, rel_table):
    x1 = np.asarray(x1, np.float32)
    x2 = np.asarray(x2, np.float32)
    rel_table = np.asarray(rel_table, np.float32)

    HH = x2.shape[2]
    residue = _interp3(_pw(x1, np.asarray(w_ch, np.float32)) +
                       np.asarray(b_ch, np.float32).reshape(1, -1, 1, 1, 1),
                       (HH, HH, HH))
    x1n = _bn(x1, np.asarray(gamma_l, np.float32), np.asarray(beta_l, np.float32))
    x2n = _bn(x2, np.asarray(gamma_h, np.float32), np.asarray(beta_h, np.float32))
    kv = _pw(_dw(x1n, np.asarray(kv_dw, np.float32)), np.asarray(kv_pw, np.float32))
    k_, v_ = kv[:, :OUT_CH], kv[:, OUT_CH:]
    q_ = _pw(_dw(x2n, np.asarray(q_dw, np.float32)), np.asarray(q_pw, np.float32))

    def heads_split(t):
        # channel c = dd*HEADS + h -> (heads, space, dim)
        t = t.reshape(DIM_HEAD, HEADS, -1)
        return np.ascontiguousarray(t.transpose(1, 2, 0))

    qh = heads_split(q_.reshape(OUT_CH, -1))
    # pre-interp per-head k/v: (4, 32, 512), layout [dim, (x,y,z) 8^3]
    ksm = np.ascontiguousarray(
        k_.reshape(DIM_HEAD, HEADS, 512).transpose(1, 0, 2))
    vsm = np.ascontiguousarray(
        v_.reshape(DIM_HEAD, HEADS, 512).transpose(1, 0, 2))

    try:
        o = _device_attention(qh, ksm, vsm, rel_table)
    except Exception as exc:  # insurance: keep output correct
        print(f"[kernel] device path failed ({exc!r}); numpy fallback",
              file=sys.stderr)
        o = _host_attention(qh, ksm, vsm, rel_table)

    # reassemble channels: o_full[dd*4+h, i] = o[h, i, dd]
    o_full = np.zeros((OUT_CH, N), np.float32)
    for h in range(HEADS):
        o_full[h::HEADS, :] = o[h].T
    o_sp = o_full.reshape(1, OUT_CH, R, R, R)

    o1 = _pw(_dw(o_sp, np.asarray(out_dw, np.float32)),
             np.asarray(out_pw, np.float32))
    o1 = o1 + residue
    res2 = o1
    o2 = np.maximum(_bn(o1, np.asarray(gamma2, np.float32),
                        np.asarray(beta2, np.float32)), 0.0)
    o3 = _pw(o2, np.asarray(w_mlp, np.float32))
    return (o3 + res2).astype(np.float32)
